# revision 19
# baseline (speedup 1.0000x reference)
"""CMRGCN Trainium2 kernel (v2).

Sharding: data-parallel over batch B=8 across the 8 NeuronCores (core b gets
batch b). Adjacency / neighbor weights / fused relation weights are replicated.

v2 changes over the v1 baseline:
  - m (node) moving dim trimmed 512 -> 500 in every matmul.
  - h / activation temps / osb / DRAM outputs in bf16 (DVE 2x tensor_tensor,
    halved output DMA); fp32 restored on host.
  - relu moved to DVE tensor_scalar (add-bias, max 0); adds split DVE/GpSimd;
    tanh + osb assembly on ACT: all four engines loaded.
  - gather (dense W_g matmul) chains interleaved into the layer loops: block 0
    (x) during layer 1, block 1 (h1) during layer 2, block 2 (h2) at the tail,
    so PE never idles on elementwise dependencies.
  - PE transposes in bf16 (1.0 cycles/row), accumulated 4 t's per PSUM bank.
  - loop order t-outer/tg-inner to keep live bf16 temps ~10 instead of ~40.

Per-core layout (N=500, padded node tiles of 128, NP=512):
  g   [4 x (128, 3, T=12, MIX=2, 64)] SBUF f32r, node-major: c-block 0 = x,
      1 = layer-1 h, 2 = layer-2 h.
  Adjacency matmul: lhsT = g-slice [n, (i,d)], rhs = adj [n, m(500)]
      -> P^T psum [(j,d), m].
  Weight matmul:    lhsT = fused-W pair block [128, 128], rhs = P^T copy
      -> preact psum [(i,d'), m]; d-path via +/-W pairs; bias in activations.
  h [128=(i,d'), (t, m)] bf16; PE-transposed (bf16) back into next g c-block.
  Final neighbor gather = dense matmul with host-densified W_g, assembled into
  bf16 osb tiles [128, m, t] and DMA'd as [64, 500, 12] contiguous blocks.
"""

import os
import numpy as np

B, T, N, DIM = 8, 12, 500, 64
N_MIX, N_LAYERS, N_HEADS, N_REL, NG, K = 2, 2, 4, 8, 2, 20
NP = 512          # padded node count
KT = NP // 128    # node tiles
C = DIM * (N_LAYERS + 1)   # 192 channels per mix in g
NCORES = 8

_BUILT = {}


def _rel(tg, i, j):
    return (tg * N_MIX + i) * N_MIX + j


def _build():
    """Build + trace the single-core SPMD Bass program once."""
    if "nc" in _BUILT:
        return _BUILT["nc"]

    from contextlib import ExitStack
    import concourse.bass as bass
    import concourse.tile as tile
    import concourse.mybir as mybir
    from concourse import bacc
    from concourse.masks import make_identity

    f32 = mybir.dt.float32
    f32r = mybir.dt.float32r
    bf16 = mybir.dt.bfloat16
    AF = mybir.ActivationFunctionType
    ALU = mybir.AluOpType

    nc = bacc.Bacc("TRN2", target_bir_lowering=False, debug=False)

    xn_d = nc.dram_tensor("xn", [N_MIX, NP, T, DIM], f32r, kind="ExternalInput").ap()
    adj_d = nc.dram_tensor("adj", [NG, NP, NP], f32r, kind="ExternalInput").ap()
    wg_d = nc.dram_tensor("wg", [NG, NP, NP], f32r, kind="ExternalInput").ap()
    # W-matmul weight blocks [128=(j,d), 128=(i,d')], K=128 with zero halves so
    # every matmul operand sits at base partition 0 (base-64 operands crash HW)
    wmm_d = nc.dram_tensor("wmm", [128, 12 * 128], f32r, kind="ExternalInput").ap()
    bias_d = nc.dram_tensor("bias", [128, 16], f32, kind="ExternalInput").ap()
    # t-major output layout: contiguous PSUM->SBUF copies and contiguous DMA;
    # host transposes [.., T, N] -> [.., N, T] for free.
    out_d = [
        nc.dram_tensor(f"out{i}", [NG * C, T, N], bf16, kind="ExternalOutput").ap()
        for i in range(N_MIX)
    ]

    with tile.TileContext(nc) as tc, ExitStack() as ctx:
        wpool = ctx.enter_context(tc.tile_pool(name="wpool", bufs=1))
        gpool = ctx.enter_context(tc.tile_pool(name="gpool", bufs=1))
        gmpool = ctx.enter_context(tc.tile_pool(name="gmpool", bufs=16))
        ptpool = ctx.enter_context(tc.tile_pool(name="ptpool", bufs=4))
        hpool = ctx.enter_context(tc.tile_pool(name="hpool", bufs=1))
        tmppool = ctx.enter_context(tc.tile_pool(name="tmppool", bufs=16))
        outpool = ctx.enter_context(tc.tile_pool(name="outpool", bufs=3))
        psA = ctx.enter_context(tc.tile_pool(name="psA", bufs=4, space="PSUM"))
        psW = ctx.enter_context(tc.tile_pool(name="psW", bufs=4, space="PSUM"))

        # --- constants / weights to SBUF ---
        wmm_sb = wpool.tile([128, 12 * 128], f32r, name="wmm_sb")
        nc.sync.dma_start(out=wmm_sb[:], in_=wmm_d[:])
        bias_sb = wpool.tile([128, 16], f32, name="bias_sb")
        nc.sync.dma_start(out=bias_sb[:], in_=bias_d[:])
        identb = wpool.tile([128, 128], bf16, name="identb")
        make_identity(nc, identb[:])

        def wmm_blk(idx):  # [128, 128] f32r lhsT, base partition 0
            return wmm_sb[:, idx * 128:(idx + 1) * 128]

        def c_wblk(l, tg, j):
            return wmm_blk((l * NG + tg) * 2 + j)

        def d_wblk(l, tg):
            return wmm_blk(8 + l * NG + tg)

        def c_bias(l, tg, j):
            col = (l * NG + tg) * 2 + j
            return bias_sb[:, col:col + 1]

        def d_bias(l, tg):
            return bias_sb[:, 8 + l * NG + tg: 8 + l * NG + tg + 1]

        def hconst(l):
            return bias_sb[:, 12 + l:12 + l + 1]

        # --- load x into g c-block 0; adjacency + gather-weight tiles ---
        g = []
        for mt in range(KT):
            gt = gpool.tile([128, 3, T, N_MIX, DIM], f32r, name=f"g{mt}", tag=f"g{mt}")
            g.append(gt)

        adj_sb, wg_sb = {}, {}

        def load_gm_tile(dst, src_d, tg, kt, key):
            tile_ = gmpool.tile([128, NP], f32r, name=f"{key}{tg}{kt}", tag="gm")
            nc.sync.dma_start(out=tile_[:], in_=src_d[tg, kt * 128:(kt + 1) * 128, :])
            dst[(tg, kt)] = tile_

        def load_x_chunk(mt, th):
            # half-t chunk for one node tile (DMA APs are limited to 3 dims,
            # so one transfer per mix)
            for i in range(N_MIX):
                nc.sync.dma_start(
                    out=g[mt][:, 0, th * 6:(th + 1) * 6, i, :],
                    in_=xn_d[i, mt * 128:(mt + 1) * 128, th * 6:(th + 1) * 6, :],
                )

        # arrival order tuned so the t=0 adjacency chain starts ~4us in: the
        # kt-th matmul of the chain needs only adj0[kt] + x[mt=kt], so
        # interleave those pairs first.
        for kt in range(KT):
            load_gm_tile(adj_sb, adj_d, 0, kt, "adj")
            load_x_chunk(kt, 0)
        for kt in range(KT):
            load_gm_tile(adj_sb, adj_d, 1, kt, "adj")
        for mt in range(KT):
            load_x_chunk(mt, 1)
        for kt in range(KT):
            load_gm_tile(wg_sb, wg_d, 0, kt, "wg")
        for kt in range(KT):
            load_gm_tile(wg_sb, wg_d, 1, kt, "wg")

        # g[3] node partitions 116:128 of c-blocks 1,2 are never written by the
        # transpose copies; zero them once so NaN SBUF garbage can't poison the
        # (zero-weighted) matmul contractions. Engine memsets reject this AP
        # (partition base 116), so DMA zeros from adj's zero-padded DRAM rows.
        g3tail = g[3][116:128, 1:3, :, :, :].rearrange("p a t i d -> p (a t i) d")
        for q in range(6):
            nc.sync.dma_start(
                out=g3tail[:, q * 8:(q + 1) * 8, :],
                in_=adj_d[0, 500:512, 0:512].rearrange("p (a d) -> p a d", a=8),
            )

        # ---------------- gather machinery (interleaved) ----------------
        osb_tiles = {}

        def gather_chain(tg, bp, t):
            if (tg, bp) not in osb_tiles:
                osb_tiles[(tg, bp)] = outpool.tile(
                    [128, T, N], bf16, name=f"osb{tg}{bp}", tag="osb")
            osb = osb_tiles[(tg, bp)]
            ps = psA.tile([128, N], f32, name="psg", tag="psa")
            for kt in range(KT):
                nc.tensor.matmul(
                    ps[:],
                    g[kt][:, bp, t, :, :],
                    wg_sb[(tg, kt)][:, 0:N],
                    start=(kt == 0), stop=(kt == KT - 1),
                )
            nc.scalar.copy(osb[:, t, :], ps[:])
            if t == T - 1:
                for i in range(N_MIX):
                    nc.sync.dma_start(
                        out=out_d[i][tg * C + bp * DIM: tg * C + (bp + 1) * DIM, :, :],
                        in_=osb[i * DIM:(i + 1) * DIM, :, :],
                    )
                del osb_tiles[(tg, bp)]

        def gather_tasks_for_block(bp):
            return [(tg, bp, t) for tg in range(NG) for t in range(T)]

        # ---------------- layers ----------------
        for l in range(N_LAYERS):
            tasks = gather_tasks_for_block(l)  # block l ready: x for l=0, h1 for l=1
            h = hpool.tile([128, T, N], bf16, name=f"h{l}", tag="h")
            for t in range(T):
                terms = {}
                for tg in range(NG):
                    # adjacency matmuls -> P^T psum [(j,d), m]
                    ps = psA.tile([128, N], f32, name="psadj", tag="psa")
                    for kt in range(KT):
                        nc.tensor.matmul(
                            ps[:],
                            g[kt][:, l, t, :, :],
                            adj_sb[(tg, kt)][:, 0:N],
                            start=(kt == 0),
                            stop=(kt == KT - 1),
                        )
                    pt = ptpool.tile([128, N], f32r, name="pt", tag="pt")
                    nc.vector.tensor_copy(pt[:], ps[:])
                    ptr = pt[:]

                    # weight matmuls; activations split DVE (j=0 relu) /
                    # ACT (j=1 relu + tanh), outputs bf16
                    for j in range(N_MIX):
                        pw = psW.tile([128, N], f32, name="psw", tag="psw")
                        nc.tensor.matmul(pw[:], c_wblk(l, tg, j), ptr,
                                         start=True, stop=True)
                        rc = tmppool.tile([128, N], bf16, name="rc", tag="tmp")
                        if j == 0:
                            nc.vector.tensor_scalar(
                                rc[:], pw[:], c_bias(l, tg, j), 0.0,
                                ALU.add, ALU.max,
                            )
                        else:
                            nc.scalar.activation(
                                rc[:], pw[:], AF.Relu, bias=c_bias(l, tg, j))
                        terms[(tg, "c", j)] = rc
                    pd = psW.tile([128, N], f32, name="psw2", tag="psw")
                    nc.tensor.matmul(pd[:], d_wblk(l, tg), ptr,
                                     start=True, stop=True)
                    dt_ = tmppool.tile([128, N], bf16, name="dt", tag="tmp")
                    nc.scalar.activation(dt_[:], pd[:], AF.Tanh, bias=d_bias(l, tg))
                    terms[(tg, "d")] = dt_

                    # interleave one gather chain per (t, tg) slot; in layer 0
                    # hold off until t>=2 so the wg DMA isn't on the PE
                    # critical path at startup
                    if tasks and (l > 0 or t >= 2):
                        gather_chain(*tasks.pop(0))

                # h[:, t, :] = sum of 6 terms + hconst: pair-sums on GpSimd
                # (SBUF-only bf16), stt + final add on DVE (2x bf16)
                a1 = tmppool.tile([128, N], bf16, name="a1", tag="tmp")
                nc.gpsimd.tensor_add(a1[:], terms[(0, "c", 0)][:], terms[(0, "c", 1)][:])
                a2 = tmppool.tile([128, N], bf16, name="a2", tag="tmp")
                nc.gpsimd.tensor_add(a2[:], terms[(1, "c", 0)][:], terms[(1, "c", 1)][:])
                a3 = tmppool.tile([128, N], bf16, name="a3", tag="tmp")
                nc.gpsimd.tensor_add(a3[:], a1[:], a2[:])
                a4 = tmppool.tile([128, N], bf16, name="a4", tag="tmp")
                nc.vector.scalar_tensor_tensor(
                    a4[:], terms[(0, "d")][:], hconst(l), a3[:],
                    op0=ALU.add, op1=ALU.add,
                )
                nc.vector.tensor_add(h[:, t, :], a4[:], terms[(1, "d")][:])

            # leftover gather chains for this block (shouldn't happen: 24 = 24)
            while tasks:
                gather_chain(*tasks.pop(0))

            # transpose h back into g c-block (l+1), 4 t's per PSUM bank:
            # [128=(i,d'), m-chunk] -> [m-chunk, (i,d')].  After layer 2,
            # go tq-outer and weave in the block-2 gather chains for the
            # just-transposed timesteps so the tail pipelines.
            def transpose_quad(mt, tq):
                lo = mt * 128
                hi = min((mt + 1) * 128, N)
                w = hi - lo
                pst = psW.tile([128, 4, 128], bf16, name="pstr", tag="psw")
                for tt in range(4):
                    t = tq * 4 + tt
                    nc.tensor.transpose(
                        pst[0:w, tt, :],
                        h[:, t, lo:hi],
                        identb[:],
                    )
                nc.vector.tensor_copy(
                    g[mt][0:w, l + 1, tq * 4:(tq + 1) * 4, :, :],
                    pst[0:w, :, :].rearrange("p q (i d) -> p q i d", i=N_MIX),
                )

            if l == 0:
                for mt in range(KT):
                    for tq in range(T // 4):
                        transpose_quad(mt, tq)
            else:
                for tq in range(T // 4):
                    for mt in range(KT):
                        transpose_quad(mt, tq)
                    for tt in range(4):
                        for tg in range(NG):
                            gather_chain(tg, 2, tq * 4 + tt)

    nc.compile()
    _BUILT["nc"] = nc
    return nc


def _host_prep(x0, x1, graphs, neighbors, neighbors_weight, a_weight, B_weight,
               a_bias, B_bias):
    """Fuse weights, densify gather, build per-core input maps."""
    f = np.float32
    x0 = np.asarray(x0, f)
    x1 = np.asarray(x1, f)
    graphs = np.asarray(graphs, f)
    neighbors = np.asarray(neighbors).astype(np.int64)
    neighbors_weight = np.asarray(neighbors_weight, f)
    a_weight = np.asarray(a_weight, f)
    B_weight = np.asarray(B_weight, f)
    a_bias = np.asarray(a_bias, f)
    B_bias = np.asarray(B_bias, f)

    # fused relation weights: wc/wd [R, L, D, D], bc/bd [R, L, D]
    wc = np.sum(a_weight[0] * B_weight, axis=1)[:, :]  # [R, L, D, D]
    wd = np.sum(a_weight[1] * B_weight, axis=1)
    bc = np.sum(a_bias[0] * B_bias, axis=1)            # [R, L, D]
    bd = np.sum(a_bias[1] * B_bias, axis=1)
    # wmm blob: 12 blocks of [128=(j,d), 128=(i,d')], K=128 with zero halves.
    # c block (l,tg,j): rows j*64.. hold [wc(tg,0,j) | wc(tg,1,j)], rest zero.
    # d block (l,tg): rows 0:64 = [-wd(r01) | +wd(r10)], rows 64:128 = [+wd(r01) | -wd(r10)]
    wmm = np.zeros((128, 12 * 128), f)
    for l in range(N_LAYERS):
        for tg in range(NG):
            for j in range(N_MIX):
                blk = (l * NG + tg) * 2 + j
                r0 = j * 64
                wmm[r0:r0 + 64, blk * 128: blk * 128 + 64] = wc[_rel(tg, 0, j), l]
                wmm[r0:r0 + 64, blk * 128 + 64: blk * 128 + 128] = wc[_rel(tg, 1, j), l]
            blk = 8 + l * NG + tg
            wd01, wd10 = wd[_rel(tg, 0, 1), l], wd[_rel(tg, 1, 0), l]
            wmm[0:64, blk * 128: blk * 128 + 64] = -wd01
            wmm[0:64, blk * 128 + 64: blk * 128 + 128] = wd10
            wmm[64:128, blk * 128: blk * 128 + 64] = wd01
            wmm[64:128, blk * 128 + 64: blk * 128 + 128] = -wd10

    bias = np.zeros((128, 16), f)
    for l in range(N_LAYERS):
        for tg in range(NG):
            for j in range(N_MIX):
                col = (l * NG + tg) * 2 + j
                bias[0:64, col] = bc[_rel(tg, 0, j), l]
                bias[64:128, col] = bc[_rel(tg, 1, j), l]
            col = 8 + l * NG + tg
            bias[0:64, col] = bd[_rel(tg, 0, 1), l]
            bias[64:128, col] = bd[_rel(tg, 1, 0), l]
        hc = np.zeros(128, f)
        for i in range(N_MIX):
            acc = np.zeros(DIM, f)
            for tg in range(NG):
                acc += np.tanh(bd[_rel(tg, i, i), l])
            hc[i * DIM:(i + 1) * DIM] = acc
        bias[:, 12 + l] = hc

    adjp = np.zeros((NG, NP, NP), f)
    adjp[:, :N, :N] = graphs
    wgp = np.zeros((NG, NP, NP), f)
    for tg in range(NG):
        np.add.at(
            wgp[tg],
            (neighbors[tg].reshape(-1),
             np.repeat(np.arange(N), K)),
            neighbors_weight[tg].reshape(-1),
        )

    in_maps = []
    for b in range(NCORES):
        xn = np.zeros((N_MIX, NP, T, DIM), f)
        xn[0, :N] = np.transpose(x0[b], (1, 2, 0))  # [D,N,T] -> [N,T,D]
        xn[1, :N] = np.transpose(x1[b], (1, 2, 0))
        in_maps.append({
            "xn": xn, "adj": adjp, "wg": wgp, "wmm": wmm, "bias": bias,
        })
    return in_maps


def kernel(x0, x1, graphs, neighbors, neighbors_weight, a_weight, B_weight,
           a_bias, B_bias):
    from concourse.bass_utils import run_bass_kernel_spmd

    nc = _build()
    in_maps = _host_prep(x0, x1, graphs, neighbors, neighbors_weight,
                         a_weight, B_weight, a_bias, B_bias)
    trace = bool(int(os.environ.get("KERNEL_TRACE", "0")))
    res = run_bass_kernel_spmd(nc, in_maps, list(range(NCORES)), trace=trace)
    kernel.last_result = res

    # device layout is [C, T, N]; swap back to [C, N, T] on host
    out0 = np.stack([np.asarray(res.results[b]["out0"], np.float32).transpose(0, 2, 1)
                     for b in range(NCORES)])  # [B, 384, 500, 12]
    out1 = np.stack([np.asarray(res.results[b]["out1"], np.float32).transpose(0, 2, 1)
                     for b in range(NCORES)])
    return out0, out1


kernel.last_result = None


# revision 21
# speedup vs baseline: 1.0332x; 1.0332x over previous
"""CMRGCN Trainium2 kernel (v2).

Sharding: data-parallel over batch B=8 across the 8 NeuronCores (core b gets
batch b). Adjacency / neighbor weights / fused relation weights are replicated.

v2 changes over the v1 baseline:
  - m (node) moving dim trimmed 512 -> 500 in every matmul.
  - h / activation temps / osb / DRAM outputs in bf16 (DVE 2x tensor_tensor,
    halved output DMA); fp32 restored on host.
  - relu moved to DVE tensor_scalar (add-bias, max 0); adds split DVE/GpSimd;
    tanh + osb assembly on ACT: all four engines loaded.
  - gather (dense W_g matmul) chains interleaved into the layer loops: block 0
    (x) during layer 1, block 1 (h1) during layer 2, block 2 (h2) at the tail,
    so PE never idles on elementwise dependencies.
  - PE transposes in bf16 (1.0 cycles/row), accumulated 4 t's per PSUM bank.
  - loop order t-outer/tg-inner to keep live bf16 temps ~10 instead of ~40.

Per-core layout (N=500, padded node tiles of 128, NP=512):
  g   [4 x (128, 3, T=12, MIX=2, 64)] SBUF f32r, node-major: c-block 0 = x,
      1 = layer-1 h, 2 = layer-2 h.
  Adjacency matmul: lhsT = g-slice [n, (i,d)], rhs = adj [n, m(500)]
      -> P^T psum [(j,d), m].
  Weight matmul:    lhsT = fused-W pair block [128, 128], rhs = P^T copy
      -> preact psum [(i,d'), m]; d-path via +/-W pairs; bias in activations.
  h [128=(i,d'), (t, m)] bf16; PE-transposed (bf16) back into next g c-block.
  Final neighbor gather = dense matmul with host-densified W_g, assembled into
  bf16 osb tiles [128, m, t] and DMA'd as [64, 500, 12] contiguous blocks.
"""

import os
import numpy as np

B, T, N, DIM = 8, 12, 500, 64
N_MIX, N_LAYERS, N_HEADS, N_REL, NG, K = 2, 2, 4, 8, 2, 20
NP = 512          # padded node count
KT = NP // 128    # node tiles
C = DIM * (N_LAYERS + 1)   # 192 channels per mix in g
NCORES = 8

_BUILT = {}


def _rel(tg, i, j):
    return (tg * N_MIX + i) * N_MIX + j


def _build():
    """Build + trace the single-core SPMD Bass program once."""
    if "nc" in _BUILT:
        return _BUILT["nc"]

    from contextlib import ExitStack
    import concourse.bass as bass
    import concourse.tile as tile
    import concourse.mybir as mybir
    from concourse import bacc
    from concourse.masks import make_identity

    f32 = mybir.dt.float32
    f32r = mybir.dt.float32r
    bf16 = mybir.dt.bfloat16
    AF = mybir.ActivationFunctionType
    ALU = mybir.AluOpType

    nc = bacc.Bacc("TRN2", target_bir_lowering=False, debug=False)

    xn_d = nc.dram_tensor("xn", [N_MIX, NP, T, DIM], f32r, kind="ExternalInput").ap()
    adj_d = nc.dram_tensor("adj", [NG, NP, NP], f32r, kind="ExternalInput").ap()
    wg_d = nc.dram_tensor("wg", [NG, NP, NP], f32r, kind="ExternalInput").ap()
    # W-matmul weight blocks [128=(j,d), 128=(i,d')], K=128 with zero halves so
    # every matmul operand sits at base partition 0 (base-64 operands crash HW)
    wmm_d = nc.dram_tensor("wmm", [128, 12 * 128], f32r, kind="ExternalInput").ap()
    bias_d = nc.dram_tensor("bias", [128, 16], f32, kind="ExternalInput").ap()
    # t-major output layout: contiguous PSUM->SBUF copies and contiguous DMA;
    # host transposes [.., T, N] -> [.., N, T] for free.
    out_d = [
        nc.dram_tensor(f"out{i}", [NG * C, T, N], bf16, kind="ExternalOutput").ap()
        for i in range(N_MIX)
    ]

    with tile.TileContext(nc) as tc, ExitStack() as ctx:
        wpool = ctx.enter_context(tc.tile_pool(name="wpool", bufs=1))
        gpool = ctx.enter_context(tc.tile_pool(name="gpool", bufs=1))
        gmpool = ctx.enter_context(tc.tile_pool(name="gmpool", bufs=16))
        ptpool = ctx.enter_context(tc.tile_pool(name="ptpool", bufs=4))
        hpool = ctx.enter_context(tc.tile_pool(name="hpool", bufs=1))
        tmppool = ctx.enter_context(tc.tile_pool(name="tmppool", bufs=16))
        outpool = ctx.enter_context(tc.tile_pool(name="outpool", bufs=3))
        psA = ctx.enter_context(tc.tile_pool(name="psA", bufs=4, space="PSUM"))
        psW = ctx.enter_context(tc.tile_pool(name="psW", bufs=4, space="PSUM"))

        # --- constants / weights to SBUF ---
        wmm_sb = wpool.tile([128, 12 * 128], f32r, name="wmm_sb")
        nc.sync.dma_start(out=wmm_sb[:], in_=wmm_d[:])
        bias_sb = wpool.tile([128, 16], f32, name="bias_sb")
        nc.sync.dma_start(out=bias_sb[:], in_=bias_d[:])
        identb = wpool.tile([128, 128], bf16, name="identb")
        make_identity(nc, identb[:])

        def wmm_blk(idx):  # [128, 128] f32r lhsT, base partition 0
            return wmm_sb[:, idx * 128:(idx + 1) * 128]

        def c_wblk(l, tg, j):
            return wmm_blk((l * NG + tg) * 2 + j)

        def d_wblk(l, tg):
            return wmm_blk(8 + l * NG + tg)

        def c_bias(l, tg, j):
            col = (l * NG + tg) * 2 + j
            return bias_sb[:, col:col + 1]

        def d_bias(l, tg):
            return bias_sb[:, 8 + l * NG + tg: 8 + l * NG + tg + 1]

        def hconst(l):
            return bias_sb[:, 12 + l:12 + l + 1]

        # --- load x into g c-block 0; adjacency + gather-weight tiles ---
        g = []
        for mt in range(KT):
            gt = gpool.tile([128, 3, T, N_MIX, DIM], f32r, name=f"g{mt}", tag=f"g{mt}")
            g.append(gt)

        adj_sb, wg_sb = {}, {}

        def load_gm_tile(dst, src_d, tg, kt, key):
            tile_ = gmpool.tile([128, NP], f32r, name=f"{key}{tg}{kt}", tag="gm")
            nc.sync.dma_start(out=tile_[:], in_=src_d[tg, kt * 128:(kt + 1) * 128, :])
            dst[(tg, kt)] = tile_

        def load_x_chunk(mt, th):
            # half-t chunk for one node tile (DMA APs are limited to 3 dims,
            # so one transfer per mix)
            for i in range(N_MIX):
                nc.sync.dma_start(
                    out=g[mt][:, 0, th * 6:(th + 1) * 6, i, :],
                    in_=xn_d[i, mt * 128:(mt + 1) * 128, th * 6:(th + 1) * 6, :],
                )

        # arrival order tuned so the t=0 adjacency chain starts ~4us in: the
        # kt-th matmul of the chain needs only adj0[kt] + x[mt=kt], so
        # interleave those pairs first.
        for kt in range(KT):
            load_gm_tile(adj_sb, adj_d, 0, kt, "adj")
            load_x_chunk(kt, 0)
        for kt in range(KT):
            load_gm_tile(adj_sb, adj_d, 1, kt, "adj")
        for mt in range(KT):
            load_x_chunk(mt, 1)
        for kt in range(KT):
            load_gm_tile(wg_sb, wg_d, 0, kt, "wg")
        for kt in range(KT):
            load_gm_tile(wg_sb, wg_d, 1, kt, "wg")

        # g[3] node partitions 116:128 of c-blocks 1,2 are never written by the
        # transpose copies; zero them once so NaN SBUF garbage can't poison the
        # (zero-weighted) matmul contractions. Engine memsets reject this AP
        # (partition base 116), so DMA zeros from adj's zero-padded DRAM rows.
        g3tail = g[3][116:128, 1:3, :, :, :].rearrange("p a t i d -> p (a t i) d")
        for q in range(6):
            nc.sync.dma_start(
                out=g3tail[:, q * 8:(q + 1) * 8, :],
                in_=adj_d[0, 500:512, 0:512].rearrange("p (a d) -> p a d", a=8),
            )

        # ---------------- gather machinery (interleaved) ----------------
        osb_tiles = {}

        def gather_chain(tg, bp, t):
            if (tg, bp) not in osb_tiles:
                osb_tiles[(tg, bp)] = outpool.tile(
                    [128, T, N], bf16, name=f"osb{tg}{bp}", tag="osb")
            osb = osb_tiles[(tg, bp)]
            ps = psA.tile([128, N], f32, name="psg", tag="psa")
            for kt in range(KT):
                nc.tensor.matmul(
                    ps[:],
                    g[kt][:, bp, t, :, :],
                    wg_sb[(tg, kt)][:, 0:N],
                    start=(kt == 0), stop=(kt == KT - 1),
                )
            nc.scalar.copy(osb[:, t, :], ps[:])
            # fire the output DMA in two t-halves so the last transfer after
            # the final matmul is only half an osb tile
            if t == T // 2 - 1 or t == T - 1:
                hlo = 0 if t < T // 2 else T // 2
                for i in range(N_MIX):
                    nc.sync.dma_start(
                        out=out_d[i][tg * C + bp * DIM: tg * C + (bp + 1) * DIM,
                                     hlo:t + 1, :],
                        in_=osb[i * DIM:(i + 1) * DIM, hlo:t + 1, :],
                    )
                if t == T - 1:
                    del osb_tiles[(tg, bp)]

        def gather_tasks_for_block(bp):
            return [(tg, bp, t) for tg in range(NG) for t in range(T)]

        # ---------------- layers ----------------
        for l in range(N_LAYERS):
            tasks = gather_tasks_for_block(l)  # block l ready: x for l=0, h1 for l=1
            h = hpool.tile([128, T, N], bf16, name=f"h{l}", tag="h")
            for t in range(T):
                terms = {}
                for tg in range(NG):
                    # adjacency matmuls -> P^T psum [(j,d), m]
                    ps = psA.tile([128, N], f32, name="psadj", tag="psa")
                    for kt in range(KT):
                        nc.tensor.matmul(
                            ps[:],
                            g[kt][:, l, t, :, :],
                            adj_sb[(tg, kt)][:, 0:N],
                            start=(kt == 0),
                            stop=(kt == KT - 1),
                        )
                    pt = ptpool.tile([128, N], f32r, name="pt", tag="pt")
                    nc.vector.tensor_copy(pt[:], ps[:])
                    ptr = pt[:]

                    # weight matmuls; activations split DVE (j=0 relu) /
                    # ACT (j=1 relu + tanh), outputs bf16
                    for j in range(N_MIX):
                        pw = psW.tile([128, N], f32, name="psw", tag="psw")
                        nc.tensor.matmul(pw[:], c_wblk(l, tg, j), ptr,
                                         start=True, stop=True)
                        rc = tmppool.tile([128, N], bf16, name="rc", tag="tmp")
                        if j == 0:
                            nc.vector.tensor_scalar(
                                rc[:], pw[:], c_bias(l, tg, j), 0.0,
                                ALU.add, ALU.max,
                            )
                        else:
                            nc.scalar.activation(
                                rc[:], pw[:], AF.Relu, bias=c_bias(l, tg, j))
                        terms[(tg, "c", j)] = rc
                    pd = psW.tile([128, N], f32, name="psw2", tag="psw")
                    nc.tensor.matmul(pd[:], d_wblk(l, tg), ptr,
                                     start=True, stop=True)
                    dt_ = tmppool.tile([128, N], bf16, name="dt", tag="tmp")
                    nc.scalar.activation(dt_[:], pd[:], AF.Tanh, bias=d_bias(l, tg))
                    terms[(tg, "d")] = dt_

                    # interleave one gather chain per (t, tg) slot; in layer 0
                    # hold off until t>=2 so the wg DMA isn't on the PE
                    # critical path at startup
                    if tasks and (l > 0 or t >= 2):
                        gather_chain(*tasks.pop(0))

                # h[:, t, :] = sum of 6 terms + hconst: pair-sums on GpSimd
                # (SBUF-only bf16), stt + final add on DVE (2x bf16)
                a1 = tmppool.tile([128, N], bf16, name="a1", tag="tmp")
                nc.gpsimd.tensor_add(a1[:], terms[(0, "c", 0)][:], terms[(0, "c", 1)][:])
                a2 = tmppool.tile([128, N], bf16, name="a2", tag="tmp")
                nc.gpsimd.tensor_add(a2[:], terms[(1, "c", 0)][:], terms[(1, "c", 1)][:])
                a3 = tmppool.tile([128, N], bf16, name="a3", tag="tmp")
                nc.gpsimd.tensor_add(a3[:], a1[:], a2[:])
                a4 = tmppool.tile([128, N], bf16, name="a4", tag="tmp")
                nc.vector.scalar_tensor_tensor(
                    a4[:], terms[(0, "d")][:], hconst(l), a3[:],
                    op0=ALU.add, op1=ALU.add,
                )
                nc.vector.tensor_add(h[:, t, :], a4[:], terms[(1, "d")][:])

            # leftover gather chains for this block (shouldn't happen: 24 = 24)
            while tasks:
                gather_chain(*tasks.pop(0))

            # transpose h back into g c-block (l+1), 4 t's per PSUM bank:
            # [128=(i,d'), m-chunk] -> [m-chunk, (i,d')].  After layer 2,
            # go tq-outer and weave in the block-2 gather chains for the
            # just-transposed timesteps so the tail pipelines.
            def transpose_quad(mt, tq):
                lo = mt * 128
                hi = min((mt + 1) * 128, N)
                w = hi - lo
                pst = psW.tile([128, 4, 128], bf16, name="pstr", tag="psw")
                for tt in range(4):
                    t = tq * 4 + tt
                    nc.tensor.transpose(
                        pst[0:w, tt, :],
                        h[:, t, lo:hi],
                        identb[:],
                    )
                nc.vector.tensor_copy(
                    g[mt][0:w, l + 1, tq * 4:(tq + 1) * 4, :, :],
                    pst[0:w, :, :].rearrange("p q (i d) -> p q i d", i=N_MIX),
                )

            for mt in range(KT):
                for tq in range(T // 4):
                    transpose_quad(mt, tq)

        # ---------------- tail gather (block 2 = h2) ----------------
        for task in gather_tasks_for_block(2):
            gather_chain(*task)

    nc.compile()
    _BUILT["nc"] = nc
    return nc


def _host_prep(x0, x1, graphs, neighbors, neighbors_weight, a_weight, B_weight,
               a_bias, B_bias):
    """Fuse weights, densify gather, build per-core input maps."""
    f = np.float32
    x0 = np.asarray(x0, f)
    x1 = np.asarray(x1, f)
    graphs = np.asarray(graphs, f)
    neighbors = np.asarray(neighbors).astype(np.int64)
    neighbors_weight = np.asarray(neighbors_weight, f)
    a_weight = np.asarray(a_weight, f)
    B_weight = np.asarray(B_weight, f)
    a_bias = np.asarray(a_bias, f)
    B_bias = np.asarray(B_bias, f)

    # fused relation weights: wc/wd [R, L, D, D], bc/bd [R, L, D]
    wc = np.sum(a_weight[0] * B_weight, axis=1)[:, :]  # [R, L, D, D]
    wd = np.sum(a_weight[1] * B_weight, axis=1)
    bc = np.sum(a_bias[0] * B_bias, axis=1)            # [R, L, D]
    bd = np.sum(a_bias[1] * B_bias, axis=1)
    # wmm blob: 12 blocks of [128=(j,d), 128=(i,d')], K=128 with zero halves.
    # c block (l,tg,j): rows j*64.. hold [wc(tg,0,j) | wc(tg,1,j)], rest zero.
    # d block (l,tg): rows 0:64 = [-wd(r01) | +wd(r10)], rows 64:128 = [+wd(r01) | -wd(r10)]
    wmm = np.zeros((128, 12 * 128), f)
    for l in range(N_LAYERS):
        for tg in range(NG):
            for j in range(N_MIX):
                blk = (l * NG + tg) * 2 + j
                r0 = j * 64
                wmm[r0:r0 + 64, blk * 128: blk * 128 + 64] = wc[_rel(tg, 0, j), l]
                wmm[r0:r0 + 64, blk * 128 + 64: blk * 128 + 128] = wc[_rel(tg, 1, j), l]
            blk = 8 + l * NG + tg
            wd01, wd10 = wd[_rel(tg, 0, 1), l], wd[_rel(tg, 1, 0), l]
            wmm[0:64, blk * 128: blk * 128 + 64] = -wd01
            wmm[0:64, blk * 128 + 64: blk * 128 + 128] = wd10
            wmm[64:128, blk * 128: blk * 128 + 64] = wd01
            wmm[64:128, blk * 128 + 64: blk * 128 + 128] = -wd10

    bias = np.zeros((128, 16), f)
    for l in range(N_LAYERS):
        for tg in range(NG):
            for j in range(N_MIX):
                col = (l * NG + tg) * 2 + j
                bias[0:64, col] = bc[_rel(tg, 0, j), l]
                bias[64:128, col] = bc[_rel(tg, 1, j), l]
            col = 8 + l * NG + tg
            bias[0:64, col] = bd[_rel(tg, 0, 1), l]
            bias[64:128, col] = bd[_rel(tg, 1, 0), l]
        hc = np.zeros(128, f)
        for i in range(N_MIX):
            acc = np.zeros(DIM, f)
            for tg in range(NG):
                acc += np.tanh(bd[_rel(tg, i, i), l])
            hc[i * DIM:(i + 1) * DIM] = acc
        bias[:, 12 + l] = hc

    adjp = np.zeros((NG, NP, NP), f)
    adjp[:, :N, :N] = graphs
    wgp = np.zeros((NG, NP, NP), f)
    for tg in range(NG):
        np.add.at(
            wgp[tg],
            (neighbors[tg].reshape(-1),
             np.repeat(np.arange(N), K)),
            neighbors_weight[tg].reshape(-1),
        )

    in_maps = []
    for b in range(NCORES):
        xn = np.zeros((N_MIX, NP, T, DIM), f)
        xn[0, :N] = np.transpose(x0[b], (1, 2, 0))  # [D,N,T] -> [N,T,D]
        xn[1, :N] = np.transpose(x1[b], (1, 2, 0))
        in_maps.append({
            "xn": xn, "adj": adjp, "wg": wgp, "wmm": wmm, "bias": bias,
        })
    return in_maps


def kernel(x0, x1, graphs, neighbors, neighbors_weight, a_weight, B_weight,
           a_bias, B_bias):
    from concourse.bass_utils import run_bass_kernel_spmd

    nc = _build()
    in_maps = _host_prep(x0, x1, graphs, neighbors, neighbors_weight,
                         a_weight, B_weight, a_bias, B_bias)
    trace = bool(int(os.environ.get("KERNEL_TRACE", "0")))
    res = run_bass_kernel_spmd(nc, in_maps, list(range(NCORES)), trace=trace)
    kernel.last_result = res

    # device layout is [C, T, N]; swap back to [C, N, T] on host
    out0 = np.stack([np.asarray(res.results[b]["out0"], np.float32).transpose(0, 2, 1)
                     for b in range(NCORES)])  # [B, 384, 500, 12]
    out1 = np.stack([np.asarray(res.results[b]["out1"], np.float32).transpose(0, 2, 1)
                     for b in range(NCORES)])
    return out0, out1


kernel.last_result = None


# revision 23
# speedup vs baseline: 1.0462x; 1.0125x over previous
"""CMRGCN Trainium2 kernel (v2).

Sharding: data-parallel over batch B=8 across the 8 NeuronCores (core b gets
batch b). Adjacency / neighbor weights / fused relation weights are replicated.

v2 changes over the v1 baseline:
  - m (node) moving dim trimmed 512 -> 500 in every matmul.
  - h / activation temps / osb / DRAM outputs in bf16 (DVE 2x tensor_tensor,
    halved output DMA); fp32 restored on host.
  - relu moved to DVE tensor_scalar (add-bias, max 0); adds split DVE/GpSimd;
    tanh + osb assembly on ACT: all four engines loaded.
  - gather (dense W_g matmul) chains interleaved into the layer loops: block 0
    (x) during layer 1, block 1 (h1) during layer 2, block 2 (h2) at the tail,
    so PE never idles on elementwise dependencies.
  - PE transposes in bf16 (1.0 cycles/row), accumulated 4 t's per PSUM bank.
  - loop order t-outer/tg-inner to keep live bf16 temps ~10 instead of ~40.

Per-core layout (N=500, padded node tiles of 128, NP=512):
  g   [4 x (128, 3, T=12, MIX=2, 64)] SBUF f32r, node-major: c-block 0 = x,
      1 = layer-1 h, 2 = layer-2 h.
  Adjacency matmul: lhsT = g-slice [n, (i,d)], rhs = adj [n, m(500)]
      -> P^T psum [(j,d), m].
  Weight matmul:    lhsT = fused-W pair block [128, 128], rhs = P^T copy
      -> preact psum [(i,d'), m]; d-path via +/-W pairs; bias in activations.
  h [128=(i,d'), (t, m)] bf16; PE-transposed (bf16) back into next g c-block.
  Final neighbor gather = dense matmul with host-densified W_g, assembled into
  bf16 osb tiles [128, m, t] and DMA'd as [64, 500, 12] contiguous blocks.
"""

import os
import numpy as np

B, T, N, DIM = 8, 12, 500, 64
N_MIX, N_LAYERS, N_HEADS, N_REL, NG, K = 2, 2, 4, 8, 2, 20
NP = 512          # padded node count
KT = NP // 128    # node tiles
C = DIM * (N_LAYERS + 1)   # 192 channels per mix in g
NCORES = 8

_BUILT = {}


def _rel(tg, i, j):
    return (tg * N_MIX + i) * N_MIX + j


def _build():
    """Build + trace the single-core SPMD Bass program once."""
    if "nc" in _BUILT:
        return _BUILT["nc"]

    from contextlib import ExitStack
    import concourse.bass as bass
    import concourse.tile as tile
    import concourse.mybir as mybir
    from concourse import bacc
    from concourse.masks import make_identity

    f32 = mybir.dt.float32
    f32r = mybir.dt.float32r
    bf16 = mybir.dt.bfloat16
    AF = mybir.ActivationFunctionType
    ALU = mybir.AluOpType

    nc = bacc.Bacc("TRN2", target_bir_lowering=False, debug=False)

    xn_d = nc.dram_tensor("xn", [N_MIX, NP, T, DIM], f32r, kind="ExternalInput").ap()
    adj_d = nc.dram_tensor("adj", [NG, NP, NP], f32r, kind="ExternalInput").ap()
    wg_d = nc.dram_tensor("wg", [NG, NP, NP], f32r, kind="ExternalInput").ap()
    # W-matmul weight blocks [128=(j,d), 128=(i,d')], K=128 with zero halves so
    # every matmul operand sits at base partition 0 (base-64 operands crash HW)
    wmm_d = nc.dram_tensor("wmm", [128, 12 * 128], f32r, kind="ExternalInput").ap()
    bias_d = nc.dram_tensor("bias", [128, 16], f32, kind="ExternalInput").ap()
    # t-major output layout: contiguous PSUM->SBUF copies and contiguous DMA;
    # host transposes [.., T, N] -> [.., N, T] for free.
    out_d = [
        nc.dram_tensor(f"out{i}", [NG * C, T, N], bf16, kind="ExternalOutput").ap()
        for i in range(N_MIX)
    ]

    with tile.TileContext(nc) as tc, ExitStack() as ctx:
        wpool = ctx.enter_context(tc.tile_pool(name="wpool", bufs=1))
        gpool = ctx.enter_context(tc.tile_pool(name="gpool", bufs=1))
        gmpool = ctx.enter_context(tc.tile_pool(name="gmpool", bufs=16))
        ptpool = ctx.enter_context(tc.tile_pool(name="ptpool", bufs=4))
        hpool = ctx.enter_context(tc.tile_pool(name="hpool", bufs=1))
        tmppool = ctx.enter_context(tc.tile_pool(name="tmppool", bufs=16))
        outpool = ctx.enter_context(tc.tile_pool(name="outpool", bufs=3))
        psA = ctx.enter_context(tc.tile_pool(name="psA", bufs=4, space="PSUM"))
        psW = ctx.enter_context(tc.tile_pool(name="psW", bufs=4, space="PSUM"))

        # --- constants / weights to SBUF ---
        wmm_sb = wpool.tile([128, 12 * 128], f32r, name="wmm_sb")
        bias_sb = wpool.tile([128, 16], f32, name="bias_sb")
        nc.sync.dma_start(out=bias_sb[:], in_=bias_d[:])
        identb = wpool.tile([128, 128], bf16, name="identb")
        make_identity(nc, identb[:])

        def wmm_blk(idx):  # [128, 128] f32r lhsT, base partition 0
            return wmm_sb[:, idx * 128:(idx + 1) * 128]

        def c_wblk(l, tg, j):
            return wmm_blk((l * NG + tg) * 2 + j)

        def d_wblk(l, tg):
            return wmm_blk(8 + l * NG + tg)

        def c_bias(l, tg, j):
            col = (l * NG + tg) * 2 + j
            return bias_sb[:, col:col + 1]

        def d_bias(l, tg):
            return bias_sb[:, 8 + l * NG + tg: 8 + l * NG + tg + 1]

        def hconst(l):
            return bias_sb[:, 12 + l:12 + l + 1]

        # --- load x into g c-block 0; adjacency + gather-weight tiles ---
        g = []
        for mt in range(KT):
            gt = gpool.tile([128, 3, T, N_MIX, DIM], f32r, name=f"g{mt}", tag=f"g{mt}")
            g.append(gt)

        adj_sb, wg_sb = {}, {}

        def load_gm(dst, src_d, tg, key):
            for kt in range(KT):
                tile_ = gmpool.tile([128, NP], f32r, name=f"{key}{tg}{kt}", tag="gm")
                nc.sync.dma_start(out=tile_[:], in_=src_d[tg, kt * 128:(kt + 1) * 128, :])
                dst[(tg, kt)] = tile_

        def load_x_chunk(tq):
            # t-slice chunk of 3 timesteps across all node tiles / mixes
            for mt in range(KT):
                for i in range(N_MIX):
                    nc.sync.dma_start(
                        out=g[mt][:, 0, tq * 3:(tq + 1) * 3, i, :],
                        in_=xn_d[i, mt * 128:(mt + 1) * 128, tq * 3:(tq + 1) * 3, :],
                    )

        # arrival order tuned so the first (t=0) adj matmuls start early; the
        # big wmm blob (0.8MB, first needed ~1us after the first adj chain)
        # goes after the first x chunk.
        load_gm(adj_sb, adj_d, 0, "adj")
        load_x_chunk(0)
        nc.sync.dma_start(out=wmm_sb[:], in_=wmm_d[:])
        load_gm(adj_sb, adj_d, 1, "adj")
        load_x_chunk(1)
        load_gm(wg_sb, wg_d, 0, "wg")
        load_x_chunk(2)
        load_gm(wg_sb, wg_d, 1, "wg")
        load_x_chunk(3)

        # warm the PE clock (HAM) with throwaway tiny matmuls while the
        # input DMAs land, so the first real matmuls run at full rate
        warmps = psW.tile([16, 16], f32, name="warm", tag="psw")
        for _ in range(72):
            nc.tensor.matmul(warmps[:], bias_sb[:, 0:16], bias_sb[:, 0:16],
                             start=True, stop=True)

        # g[3] node partitions 116:128 of c-blocks 1,2 are never written by the
        # transpose copies; zero them once so NaN SBUF garbage can't poison the
        # (zero-weighted) matmul contractions. Engine memsets reject this AP
        # (partition base 116), so DMA zeros from adj's zero-padded DRAM rows.
        g3tail = g[3][116:128, 1:3, :, :, :].rearrange("p a t i d -> p (a t i) d")
        for q in range(6):
            nc.sync.dma_start(
                out=g3tail[:, q * 8:(q + 1) * 8, :],
                in_=adj_d[0, 500:512, 0:512].rearrange("p (a d) -> p a d", a=8),
            )

        # ---------------- gather machinery (interleaved) ----------------
        osb_tiles = {}

        def gather_chain(tg, bp, t):
            if (tg, bp) not in osb_tiles:
                osb_tiles[(tg, bp)] = outpool.tile(
                    [128, T, N], bf16, name=f"osb{tg}{bp}", tag="osb")
            osb = osb_tiles[(tg, bp)]
            ps = psA.tile([128, N], f32, name="psg", tag="psa")
            for kt in range(KT):
                nc.tensor.matmul(
                    ps[:],
                    g[kt][:, bp, t, :, :],
                    wg_sb[(tg, kt)][:, 0:N],
                    start=(kt == 0), stop=(kt == KT - 1),
                )
            nc.scalar.copy(osb[:, t, :], ps[:])
            # fire the output DMA in two t-halves so the last transfer after
            # the final matmul is only half an osb tile
            if t == T // 2 - 1 or t == T - 1:
                hlo = 0 if t < T // 2 else T // 2
                for i in range(N_MIX):
                    nc.sync.dma_start(
                        out=out_d[i][tg * C + bp * DIM: tg * C + (bp + 1) * DIM,
                                     hlo:t + 1, :],
                        in_=osb[i * DIM:(i + 1) * DIM, hlo:t + 1, :],
                    )
                if t == T - 1:
                    del osb_tiles[(tg, bp)]

        def gather_tasks_for_block(bp):
            return [(tg, bp, t) for tg in range(NG) for t in range(T)]

        # ---------------- layers ----------------
        for l in range(N_LAYERS):
            tasks = gather_tasks_for_block(l)  # block l ready: x for l=0, h1 for l=1
            h = hpool.tile([128, T, N], bf16, name=f"h{l}", tag="h")
            for t in range(T):
                terms = {}
                for tg in range(NG):
                    # adjacency matmuls -> P^T psum [(j,d), m]
                    ps = psA.tile([128, N], f32, name="psadj", tag="psa")
                    for kt in range(KT):
                        nc.tensor.matmul(
                            ps[:],
                            g[kt][:, l, t, :, :],
                            adj_sb[(tg, kt)][:, 0:N],
                            start=(kt == 0),
                            stop=(kt == KT - 1),
                        )
                    pt = ptpool.tile([128, N], f32r, name="pt", tag="pt")
                    nc.vector.tensor_copy(pt[:], ps[:])
                    ptr = pt[:]

                    # weight matmuls; activations split DVE (j=0 relu) /
                    # ACT (j=1 relu + tanh), outputs bf16
                    for j in range(N_MIX):
                        pw = psW.tile([128, N], f32, name="psw", tag="psw")
                        nc.tensor.matmul(pw[:], c_wblk(l, tg, j), ptr,
                                         start=True, stop=True)
                        rc = tmppool.tile([128, N], bf16, name="rc", tag="tmp")
                        if j == 0:
                            nc.vector.tensor_scalar(
                                rc[:], pw[:], c_bias(l, tg, j), 0.0,
                                ALU.add, ALU.max,
                            )
                        else:
                            nc.scalar.activation(
                                rc[:], pw[:], AF.Relu, bias=c_bias(l, tg, j))
                        terms[(tg, "c", j)] = rc
                    pd = psW.tile([128, N], f32, name="psw2", tag="psw")
                    nc.tensor.matmul(pd[:], d_wblk(l, tg), ptr,
                                     start=True, stop=True)
                    dt_ = tmppool.tile([128, N], bf16, name="dt", tag="tmp")
                    nc.scalar.activation(dt_[:], pd[:], AF.Tanh, bias=d_bias(l, tg))
                    terms[(tg, "d")] = dt_

                    # interleave one gather chain per (t, tg) slot; in layer 0
                    # hold off until t>=2 so the wg DMA isn't on the PE
                    # critical path at startup
                    if tasks and (l > 0 or t >= 2):
                        gather_chain(*tasks.pop(0))

                # h[:, t, :] = sum of 6 terms + hconst: pair-sums on GpSimd
                # (SBUF-only bf16), stt + final add on DVE (2x bf16)
                a1 = tmppool.tile([128, N], bf16, name="a1", tag="tmp")
                nc.gpsimd.tensor_add(a1[:], terms[(0, "c", 0)][:], terms[(0, "c", 1)][:])
                a2 = tmppool.tile([128, N], bf16, name="a2", tag="tmp")
                nc.gpsimd.tensor_add(a2[:], terms[(1, "c", 0)][:], terms[(1, "c", 1)][:])
                a3 = tmppool.tile([128, N], bf16, name="a3", tag="tmp")
                nc.gpsimd.tensor_add(a3[:], a1[:], a2[:])
                a4 = tmppool.tile([128, N], bf16, name="a4", tag="tmp")
                nc.vector.scalar_tensor_tensor(
                    a4[:], terms[(0, "d")][:], hconst(l), a3[:],
                    op0=ALU.add, op1=ALU.add,
                )
                nc.vector.tensor_add(h[:, t, :], a4[:], terms[(1, "d")][:])

            # leftover gather chains for this block (shouldn't happen: 24 = 24)
            while tasks:
                gather_chain(*tasks.pop(0))

            # transpose h back into g c-block (l+1), 4 t's per PSUM bank:
            # [128=(i,d'), m-chunk] -> [m-chunk, (i,d')].  After layer 2,
            # go tq-outer and weave in the block-2 gather chains for the
            # just-transposed timesteps so the tail pipelines.
            def transpose_quad(mt, tq):
                lo = mt * 128
                hi = min((mt + 1) * 128, N)
                w = hi - lo
                pst = psW.tile([128, 4, 128], bf16, name="pstr", tag="psw")
                for tt in range(4):
                    t = tq * 4 + tt
                    nc.tensor.transpose(
                        pst[0:w, tt, :],
                        h[:, t, lo:hi],
                        identb[:],
                    )
                nc.vector.tensor_copy(
                    g[mt][0:w, l + 1, tq * 4:(tq + 1) * 4, :, :],
                    pst[0:w, :, :].rearrange("p q (i d) -> p q i d", i=N_MIX),
                )

            for mt in range(KT):
                for tq in range(T // 4):
                    transpose_quad(mt, tq)

        # ---------------- tail gather (block 2 = h2) ----------------
        for task in gather_tasks_for_block(2):
            gather_chain(*task)

    nc.compile()
    _BUILT["nc"] = nc
    return nc


def _host_prep(x0, x1, graphs, neighbors, neighbors_weight, a_weight, B_weight,
               a_bias, B_bias):
    """Fuse weights, densify gather, build per-core input maps."""
    f = np.float32
    x0 = np.asarray(x0, f)
    x1 = np.asarray(x1, f)
    graphs = np.asarray(graphs, f)
    neighbors = np.asarray(neighbors).astype(np.int64)
    neighbors_weight = np.asarray(neighbors_weight, f)
    a_weight = np.asarray(a_weight, f)
    B_weight = np.asarray(B_weight, f)
    a_bias = np.asarray(a_bias, f)
    B_bias = np.asarray(B_bias, f)

    # fused relation weights: wc/wd [R, L, D, D], bc/bd [R, L, D]
    wc = np.sum(a_weight[0] * B_weight, axis=1)[:, :]  # [R, L, D, D]
    wd = np.sum(a_weight[1] * B_weight, axis=1)
    bc = np.sum(a_bias[0] * B_bias, axis=1)            # [R, L, D]
    bd = np.sum(a_bias[1] * B_bias, axis=1)
    # wmm blob: 12 blocks of [128=(j,d), 128=(i,d')], K=128 with zero halves.
    # c block (l,tg,j): rows j*64.. hold [wc(tg,0,j) | wc(tg,1,j)], rest zero.
    # d block (l,tg): rows 0:64 = [-wd(r01) | +wd(r10)], rows 64:128 = [+wd(r01) | -wd(r10)]
    wmm = np.zeros((128, 12 * 128), f)
    for l in range(N_LAYERS):
        for tg in range(NG):
            for j in range(N_MIX):
                blk = (l * NG + tg) * 2 + j
                r0 = j * 64
                wmm[r0:r0 + 64, blk * 128: blk * 128 + 64] = wc[_rel(tg, 0, j), l]
                wmm[r0:r0 + 64, blk * 128 + 64: blk * 128 + 128] = wc[_rel(tg, 1, j), l]
            blk = 8 + l * NG + tg
            wd01, wd10 = wd[_rel(tg, 0, 1), l], wd[_rel(tg, 1, 0), l]
            wmm[0:64, blk * 128: blk * 128 + 64] = -wd01
            wmm[0:64, blk * 128 + 64: blk * 128 + 128] = wd10
            wmm[64:128, blk * 128: blk * 128 + 64] = wd01
            wmm[64:128, blk * 128 + 64: blk * 128 + 128] = -wd10

    bias = np.zeros((128, 16), f)
    for l in range(N_LAYERS):
        for tg in range(NG):
            for j in range(N_MIX):
                col = (l * NG + tg) * 2 + j
                bias[0:64, col] = bc[_rel(tg, 0, j), l]
                bias[64:128, col] = bc[_rel(tg, 1, j), l]
            col = 8 + l * NG + tg
            bias[0:64, col] = bd[_rel(tg, 0, 1), l]
            bias[64:128, col] = bd[_rel(tg, 1, 0), l]
        hc = np.zeros(128, f)
        for i in range(N_MIX):
            acc = np.zeros(DIM, f)
            for tg in range(NG):
                acc += np.tanh(bd[_rel(tg, i, i), l])
            hc[i * DIM:(i + 1) * DIM] = acc
        bias[:, 12 + l] = hc

    adjp = np.zeros((NG, NP, NP), f)
    adjp[:, :N, :N] = graphs
    wgp = np.zeros((NG, NP, NP), f)
    for tg in range(NG):
        np.add.at(
            wgp[tg],
            (neighbors[tg].reshape(-1),
             np.repeat(np.arange(N), K)),
            neighbors_weight[tg].reshape(-1),
        )

    in_maps = []
    for b in range(NCORES):
        xn = np.zeros((N_MIX, NP, T, DIM), f)
        xn[0, :N] = np.transpose(x0[b], (1, 2, 0))  # [D,N,T] -> [N,T,D]
        xn[1, :N] = np.transpose(x1[b], (1, 2, 0))
        in_maps.append({
            "xn": xn, "adj": adjp, "wg": wgp, "wmm": wmm, "bias": bias,
        })
    return in_maps


def kernel(x0, x1, graphs, neighbors, neighbors_weight, a_weight, B_weight,
           a_bias, B_bias):
    from concourse.bass_utils import run_bass_kernel_spmd

    nc = _build()
    in_maps = _host_prep(x0, x1, graphs, neighbors, neighbors_weight,
                         a_weight, B_weight, a_bias, B_bias)
    trace = bool(int(os.environ.get("KERNEL_TRACE", "0")))
    res = run_bass_kernel_spmd(nc, in_maps, list(range(NCORES)), trace=trace)
    kernel.last_result = res

    # device layout is [C, T, N]; swap back to [C, N, T] on host
    out0 = np.stack([np.asarray(res.results[b]["out0"], np.float32).transpose(0, 2, 1)
                     for b in range(NCORES)])  # [B, 384, 500, 12]
    out1 = np.stack([np.asarray(res.results[b]["out1"], np.float32).transpose(0, 2, 1)
                     for b in range(NCORES)])
    return out0, out1


kernel.last_result = None
